# revision 1
# baseline (speedup 1.0000x reference)
"""Trainium2 Bass kernel for nn_CubicModelLarge (3-layer cubic-feature MLP).

Strategy: tensor-parallel over the cubic multiplier index i (64 values, 8 per
core).  The cubic expansion is never materialized.  Per layer:

  y[b,o] = W_lin@x + b + sum_t W_sq[o,t] xsq[b,t] + sum_i x[b,i] sum_t W_cu[o,i,t] xsq[b,t]

Rewritten per core c (i in I_c = [8c, 8c+8)):

  H[b,(il,o)] = sum_J F[J,b] * Wcub[J,(il,o)]     (one f32r GEMM, J = 2176 rows)
  y_c[b,o]    = lin[b,o] + sum_il xmac[b,il] * H[b,(il,o)]
  y = AllReduce_c(y_c)

F rows: 2048 rotation products x_a*x_{(a+d)%64} (d=0..31), 64 x rows (carries
the symmetrized W_sq fold, sharded over i via the x_i scaling), 64 gap-32
products (halved).  Rotated copies of xT are built with PE selection matmuls;
products on DVE; the i-contraction is fused scalar_tensor_tensor MACs with
per-partition scalars.  Final layer partials are summed on the host.
"""

import numpy as np

D = 64
B = 1024
NCORES = 8
I_PER = D // NCORES          # 8
OUTS = (64, 64, 10)
NKCHUNK = 16                 # rotation chunks (d pairs)
NB = B // 128                # 8 batch chunks

_CACHE = {}


# ---------------------------------------------------------------- host prep --

def _maps():
    iu, ju = np.triu_indices(D)
    tmap = np.zeros((D, D), np.int64)
    tmap[iu, ju] = np.arange(len(iu))
    tmap[ju, iu] = tmap[iu, ju]
    p = np.arange(128)
    rows_t = np.zeros((NKCHUNK, 128), np.int64)
    for k in range(NKCHUNK):
        d = 2 * k + p // 64
        a = p % 64
        rows_t[k] = tmap[a, (a + d) % D]
    d32_t = tmap[np.arange(D), (np.arange(D) + 32) % D]
    return tmap, rows_t, d32_t


def _prep_layer(W, b, out):
    """-> (wcub [NCORES](2176, I_PER*out), wlin [NCORES](65, out))"""
    _, rows_t, d32_t = _maps()
    W_lin = W[:, :D]
    W_sq = W[:, D:D + 2080]
    W_cu = W[:, D + 2080:].reshape(out, D, 2080)

    iu, ju = np.triu_indices(D)
    w2 = np.zeros((out, D, D), np.float32)
    half = np.where(iu == ju, 1.0, 0.5).astype(np.float32)
    w2[:, iu, ju] = W_sq * half
    w2[:, ju, iu] = W_sq * half

    rt = rows_t.reshape(-1)
    wcubs, wlins = [], []
    for core in range(NCORES):
        I = np.arange(core * I_PER, (core + 1) * I_PER)
        M = I_PER * out
        wcub = np.zeros((17 * 128, M), np.float32)
        blk = W_cu[:, I, :][:, :, rt]                       # (out, I_PER, 2048)
        wcub[:2048] = blk.transpose(2, 1, 0).reshape(2048, M)
        w2blk = w2[:, I, :]                                 # (out, I_PER, 64)
        wcub[2048:2048 + D] = w2blk.transpose(2, 1, 0).reshape(D, M)
        d32blk = W_cu[:, I, :][:, :, d32_t] / 2
        wcub[2048 + D:] = d32blk.transpose(2, 1, 0).reshape(D, M)
        wcubs.append(np.ascontiguousarray(wcub))

        wl = np.zeros((65, out), np.float32)
        if core == 0:
            wl[:D] = W_lin.T
            wl[D] = b
        wlins.append(wl)
    return wcubs, wlins


def _sel_consts():
    """Selection matrices, concatenated (64, (NKCHUNK+2)*128).

    slot k in 0..15: [rot_{2k}; rot_{2k+1}]   sel[c, k*128 + h*64 + a] = (c == (a + 2k + h) % 64)
    slot 16: [rot0; rot0]  (builds xT2)
    slot 17: [rot32; rot32] (first 64 cols used, builds xd32)
    """
    sel = np.zeros((D, (NKCHUNK + 2) * 128), np.float32)
    for k in range(NKCHUNK):
        for p in range(128):
            d = 2 * k + p // 64
            a = p % 64
            sel[(a + d) % D, k * 128 + p] = 1.0
    for p in range(128):
        sel[p % 64, NKCHUNK * 128 + p] = 1.0
        sel[(p % 64 + 32) % D, (NKCHUNK + 1) * 128 + p] = 1.0
    return sel


# ------------------------------------------------------------------ builder --

def _build_module():
    import concourse.bacc as bacc
    import concourse.mybir as mybir
    import concourse.tile as tile

    F32 = mybir.dt.float32
    F32R = mybir.dt.float32r
    MULT = mybir.AluOpType.mult
    ADD = mybir.AluOpType.add

    nc = bacc.Bacc("TRN2", target_bir_lowering=False, num_devices=NCORES, debug=False)

    x_in = nc.dram_tensor("x", [B, D], F32, kind="ExternalInput")
    wcub_in = [
        nc.dram_tensor(f"wcub{li}", [17 * 128, I_PER * OUTS[li]], F32, kind="ExternalInput")
        for li in range(3)
    ]
    wlin_in = [
        nc.dram_tensor(f"wlin{li}", [65, OUTS[li]], F32, kind="ExternalInput")
        for li in range(3)
    ]
    colsel_in = nc.dram_tensor("colsel", [D, I_PER], F32, kind="ExternalInput")
    out_ext = nc.dram_tensor("out", [B, OUTS[2]], F32, kind="ExternalOutput")

    sel_c = nc.inline_tensor(_sel_consts(), name="selc")
    ident_c = nc.inline_tensor(np.eye(128, dtype=np.float32), name="identc")

    with tile.TileContext(nc) as tc:
        with (
            tc.tile_pool(name="wpool", bufs=2) as wpool,
            tc.tile_pool(name="spool", bufs=1) as spool,
            tc.tile_pool(name="xpool", bufs=2) as xpool,
            tc.tile_pool(name="qpool", bufs=1) as qpool,
            tc.tile_pool(name="ypool", bufs=2) as ypool,
            tc.tile_pool(name="ps_rep", bufs=2, space="PSUM") as ps_rep,
            tc.tile_pool(name="ps_h", bufs=3, space="PSUM") as ps_h,
            tc.tile_pool(name="ps_small", bufs=3, space="PSUM") as ps_small,
            tc.tile_pool(name="dpool", bufs=2, space="DRAM") as dpool,
        ):
            sel_sb = spool.tile([D, (NKCHUNK + 2) * 128], F32R, tag="sel")
            nc.sync.dma_start(sel_sb[:], sel_c.ap().bitcast(F32R))
            ident_sb = spool.tile([128, 128], F32, tag="ident")
            nc.sync.dma_start(ident_sb[:], ident_c.ap())
            colsel_sb = spool.tile([D, I_PER], F32R, tag="colsel")
            nc.sync.dma_start(colsel_sb[:], colsel_in.ap().bitcast(F32R))

            HB = 512            # half-batch
            NBH = HB // 128     # 4 chunks per half

            # per-layer weight tiles (DMA'd up front; wpool bufs=2 double-buffers)
            weights = []
            for li in range(3):
                M = I_PER * OUTS[li]
                wcub_sb = wpool.tile([128, NKCHUNK, M], F32R, tag="wcub")
                nc.sync.dma_start(
                    wcub_sb[:],
                    wcub_in[li].ap().bitcast(F32R)[: 16 * 128, :]
                    .rearrange("(k p) m -> p k m", p=128),
                )
                wx_sb = wpool.tile([D, M], F32R, tag="wx")
                nc.sync.dma_start(wx_sb[:], wcub_in[li].ap().bitcast(F32R)[2048:2048 + D, :])
                wd32_sb = wpool.tile([D, M], F32R, tag="wd32")
                nc.sync.dma_start(wd32_sb[:], wcub_in[li].ap().bitcast(F32R)[2048 + D:, :])
                wlin_sb = wpool.tile([65, OUTS[li]], F32R, tag="wlin")
                nc.sync.dma_start(wlin_sb[:], wlin_in[li].ap().bitcast(F32R))
                weights.append((wcub_sb, wx_sb, wd32_sb, wlin_sb))

            # x tiles for layer 0, both halves, straight from the input
            x_half = []
            for h in range(2):
                xs = xpool.tile([128, NBH, D], F32, tag=f"x{h}")
                nc.sync.dma_start(
                    xs[:],
                    x_in.ap()[h * HB:(h + 1) * HB, :]
                    .rearrange("(bc p) f -> p bc f", p=128),
                )
                x_half.append(xs)

            for li in range(3):
                out_l = OUTS[li]
                M = I_PER * out_l
                last = li == 2
                wcub_sb, wx_sb, wd32_sb, wlin_sb = weights[li]
                next_x = [None, None]

                for h in range(2):
                    x_sb = x_half[h]

                    # -- phase A
                    xT_sb = xpool.tile([65, HB], F32R, tag=f"xT{h}")
                    for bc in range(NBH):
                        xTp = ps_small.tile([D, 128], F32, tag="small")
                        nc.tensor.transpose(xTp[:], x_sb[:, bc, :], ident_sb[:])
                        nc.scalar.copy(xT_sb[0:D, bc * 128:(bc + 1) * 128], xTp[:])
                    nc.vector.memset(xT_sb[D:65, :].bitcast(F32), 1.0)

                    xT2_sb = xpool.tile([128, HB], F32, tag=f"xT2{h}")
                    rep00 = ps_rep.tile([128, HB], F32, tag="rep")
                    nc.tensor.matmul(
                        rep00[:], sel_sb[:, NKCHUNK * 128:(NKCHUNK + 1) * 128],
                        xT_sb[0:D, :], start=True, stop=True,
                    )
                    nc.scalar.copy(xT2_sb[:], rep00[:])

                    xd32_sb = xpool.tile([D, HB], F32R, tag=f"xd32{h}")
                    rep32 = ps_rep.tile([128, HB], F32, tag="rep")
                    nc.tensor.matmul(
                        rep32[:], sel_sb[:, (NKCHUNK + 1) * 128:(NKCHUNK + 2) * 128],
                        xT_sb[0:D, :], start=True, stop=True,
                    )
                    nc.vector.tensor_mul(xd32_sb[:], xT2_sb[0:D, :], rep32[0:D, :])

                    # -- phase B
                    xsq = []
                    for k in range(NKCHUNK):
                        rep = ps_rep.tile([128, HB], F32, tag="rep")
                        nc.tensor.matmul(
                            rep[:], sel_sb[:, k * 128:(k + 1) * 128],
                            xT_sb[0:D, :], start=True, stop=True,
                        )
                        xq = qpool.tile([128, HB], F32R, tag=f"xsq{k}h{h}")
                        nc.vector.tensor_mul(xq[:], xT2_sb[:], rep[:])
                        xsq.append(xq)

                    # -- phase C
                    y_sb = ypool.tile([128, NBH, out_l], F32, tag=f"y{h}")
                    if not last:
                        for bc in range(NBH):
                            bs = slice(bc * 128, (bc + 1) * 128)
                            h_ps = ps_h.tile([128, M], F32, tag="h")
                            for k in range(NKCHUNK):
                                nc.tensor.matmul(
                                    h_ps[:], xsq[k][:, bs], wcub_sb[:, k, :],
                                    start=(k == 0), stop=False,
                                )
                            nc.tensor.matmul(h_ps[:], xT_sb[0:D, bs], wx_sb[:], start=False, stop=False)
                            nc.tensor.matmul(h_ps[:], xd32_sb[:, bs], wd32_sb[:], start=False, stop=True)

                            lin_ps = ps_small.tile([128, out_l], F32, tag="small")
                            nc.tensor.matmul(lin_ps[:], xT_sb[0:65, bs], wlin_sb[:], start=True, stop=True)
                            xmac_ps = ps_small.tile([128, I_PER], F32, tag="small")
                            nc.tensor.matmul(xmac_ps[:], xT_sb[0:D, bs], colsel_sb[:], start=True, stop=True)
                            xmac_sb = ypool.tile([128, I_PER], F32, tag="xmac")
                            nc.scalar.copy(xmac_sb[:], xmac_ps[:])

                            nc.scalar.copy(y_sb[:, bc, :], lin_ps[:])
                            for il in range(I_PER):
                                nc.vector.scalar_tensor_tensor(
                                    y_sb[:, bc, :],
                                    h_ps[:, il * out_l:(il + 1) * out_l],
                                    xmac_sb[:, il:il + 1],
                                    y_sb[:, bc, :],
                                    op0=MULT, op1=ADD,
                                )

                        # -- phase D: AllReduce this half
                        y_bounce = dpool.tile([HB, out_l], F32, tag=f"ybounce{h}")
                        y_red = dpool.tile([HB, out_l], F32, tag=f"yred{h}")
                        nc.sync.dma_start(
                            y_bounce[:].rearrange("(bc p) o -> p bc o", p=128), y_sb[:]
                        )
                        nc.gpsimd.collective_compute(
                            "AllReduce",
                            ADD,
                            replica_groups=[list(range(NCORES))],
                            ins=[y_bounce.opt()],
                            outs=[y_red.opt()],
                        )
                        xs = xpool.tile([128, NBH, D], F32, tag=f"x{h}")
                        nc.sync.dma_start(
                            xs[:], y_red[:].rearrange("(bc p) f -> p bc f", p=128)
                        )
                        next_x[h] = xs
                    else:
                        # layer 2: stationary-W GEMM, transpose, MAC
                        h_ps = ps_h.tile([M, HB], F32, tag="h")
                        for k in range(NKCHUNK):
                            nc.tensor.matmul(
                                h_ps[:], wcub_sb[:, k, :], xsq[k][:],
                                start=(k == 0), stop=False,
                            )
                        nc.tensor.matmul(h_ps[:], wx_sb[:], xT_sb[0:D, :], start=False, stop=False)
                        nc.tensor.matmul(h_ps[:], wd32_sb[:], xd32_sb[:], start=False, stop=True)
                        h2_sb = ypool.tile([M, HB], F32, tag=f"h2{h}")
                        nc.scalar.copy(h2_sb[:], h_ps[:])

                        for bc in range(NBH):
                            bs = slice(bc * 128, (bc + 1) * 128)
                            h2t_ps = ps_small.tile([128, M], F32, tag="small")
                            nc.tensor.transpose(h2t_ps[:], h2_sb[:, bs], ident_sb[0:M, 0:M])

                            lin_ps = ps_small.tile([128, out_l], F32, tag="small")
                            nc.tensor.matmul(lin_ps[:], xT_sb[0:65, bs], wlin_sb[:], start=True, stop=True)
                            xmac_ps = ps_small.tile([128, I_PER], F32, tag="small")
                            nc.tensor.matmul(xmac_ps[:], xT_sb[0:D, bs], colsel_sb[:], start=True, stop=True)
                            xmac_sb = ypool.tile([128, I_PER], F32, tag="xmac")
                            nc.scalar.copy(xmac_sb[:], xmac_ps[:])

                            nc.scalar.copy(y_sb[:, bc, :], lin_ps[:])
                            for il in range(I_PER):
                                nc.vector.scalar_tensor_tensor(
                                    y_sb[:, bc, :],
                                    h2t_ps[:, il * out_l:(il + 1) * out_l],
                                    xmac_sb[:, il:il + 1],
                                    y_sb[:, bc, :],
                                    op0=MULT, op1=ADD,
                                )

                        nc.sync.dma_start(
                            out_ext.ap()[h * HB:(h + 1) * HB, :]
                            .rearrange("(bc p) o -> p bc o", p=128),
                            y_sb[:],
                        )

                if not last:
                    x_half = next_x

    nc.compile()
    return nc


# ------------------------------------------------------------------- runner --

def kernel(x, W0, b0, W1, b1, W2, b2):
    from concourse.bass_utils import run_bass_kernel_spmd

    if "nc" not in _CACHE:
        _CACHE["nc"] = _build_module()
    nc = _CACHE["nc"]

    x = np.ascontiguousarray(np.asarray(x, np.float32))
    Ws = [np.asarray(W, np.float32) for W in (W0, W1, W2)]
    bs = [np.asarray(b_, np.float32) for b_ in (b0, b1, b2)]

    wcubs, wlins = {}, {}
    for li in range(3):
        wcubs[li], wlins[li] = _prep_layer(Ws[li], bs[li], OUTS[li])

    in_maps = []
    for core in range(NCORES):
        I = np.arange(core * I_PER, (core + 1) * I_PER)
        colsel = np.zeros((D, I_PER), np.float32)
        colsel[I, np.arange(I_PER)] = 1.0
        m = {"x": x, "colsel": colsel}
        for li in range(3):
            m[f"wcub{li}"] = wcubs[li][core]
            m[f"wlin{li}"] = wlins[li][core]
        in_maps.append(m)

    res = run_bass_kernel_spmd(nc, in_maps, core_ids=list(range(NCORES)))
    out = np.zeros((B, OUTS[2]), np.float32)
    for core in range(NCORES):
        out += res.results[core]["out"]
    return out



# revision 8
# speedup vs baseline: 1.2175x; 1.2175x over previous
"""Trainium2 Bass kernel for nn_CubicModelLarge (3-layer cubic-feature MLP).

Strategy: tensor-parallel over the cubic multiplier index i (64 values, 8 per
core).  The cubic expansion is never materialized.  Per layer:

  y[b,o] = W_lin@x + b + sum_t W_sq[o,t] xsq[b,t] + sum_i x[b,i] sum_t W_cu[o,i,t] xsq[b,t]

Rewritten per core c (i in I_c = [8c, 8c+8)):

  H[b,(il,o)] = sum_J F[J,b] * Wcub[J,(il,o)]     (one f32r GEMM, J = 2176 rows)
  y_c[b,o]    = lin[b,o] + sum_il xmac[b,il] * H[b,(il,o)]
  y = AllReduce_c(y_c)

F rows use the sum-square basis: instead of products x_a*x_b, each row is
u = (x_a+x_b)^2/2, built by a selection-SUM matmul on the PE (two 1s per
column) followed by a Square activation on the Scalar engine (PSUM->SBUF).
This removes all DVE tensor_mul product work.  The weight fold
x_a*x_b = u_ab - x_a^2/2 - x_b^2/2 is applied host-side (corrections land on
the d=0 rows, whose value is now 2*x_a^2).

A tiny warm-up AllReduce issues first to absorb collective-init latency.
Final layer partials are summed on the host.
"""

import numpy as np

D = 64
B = 1024
NCORES = 8
I_PER = D // NCORES          # 8
OUTS = (64, 64, 10)
NKCHUNK = 16                 # rotation chunks (d pairs)
NB = B // 128                # 8 batch chunks
INV_SQRT2 = 0.7071067811865476

_CACHE = {}


# ---------------------------------------------------------------- host prep --

def _maps():
    iu, ju = np.triu_indices(D)
    tmap = np.zeros((D, D), np.int64)
    tmap[iu, ju] = np.arange(len(iu))
    tmap[ju, iu] = tmap[iu, ju]
    p = np.arange(128)
    rows_t = np.zeros((NKCHUNK, 128), np.int64)
    for k in range(NKCHUNK):
        d = 2 * k + p // 64
        a = p % 64
        rows_t[k] = tmap[a, (a + d) % D]
    d32_t = tmap[np.arange(D), (np.arange(D) + 32) % D]
    return tmap, rows_t, d32_t


def _u_transform():
    """B.T for the sum-square basis change on the 2176-row F basis.

    Rows 0..2047: rotation products (k = r//128, p = r%128, d = 2k + p//64,
    a = p%64, b = (a+d)%64).  Rows 2048..2111: x rows.  Rows 2112..2175:
    d32 products.  Old row value x_a*x_b = u_r - u_{d0(a)}/4 - u_{d0(b)}/4
    (a != b); d0 rows (a == b): x_a^2 = u_r/2.  d0(a) = row a.
    """
    n = 17 * 128
    Bm = np.zeros((n, n), np.float32)
    for r in range(2048):
        k, p = divmod(r, 128)
        d = 2 * k + p // 64
        a = p % 64
        b = (a + d) % D
        if a == b:
            Bm[r, r] = 0.5
        else:
            Bm[r, r] = 1.0
            Bm[r, a] -= 0.25
            Bm[r, b] -= 0.25
    for r in range(2048, 2048 + D):
        Bm[r, r] = 1.0
    for r in range(2048 + D, n):
        a = r - (2048 + D)
        b = (a + 32) % D
        Bm[r, r] = 1.0
        Bm[r, a] -= 0.25
        Bm[r, b] -= 0.25
    return Bm.T.copy()


def _prep_layer(W, b, out, BT):
    """-> (wcub [NCORES](2176, I_PER*out), wlin [NCORES](65, out))"""
    _, rows_t, d32_t = _maps()
    W_lin = W[:, :D]
    W_sq = W[:, D:D + 2080]
    W_cu = W[:, D + 2080:].reshape(out, D, 2080)

    iu, ju = np.triu_indices(D)
    w2 = np.zeros((out, D, D), np.float32)
    half = np.where(iu == ju, 1.0, 0.5).astype(np.float32)
    w2[:, iu, ju] = W_sq * half
    w2[:, ju, iu] = W_sq * half

    rt = rows_t.reshape(-1)
    wcubs, wlins = [], []
    for core in range(NCORES):
        I = np.arange(core * I_PER, (core + 1) * I_PER)
        M = I_PER * out
        wcub = np.zeros((17 * 128, M), np.float32)
        blk = W_cu[:, I, :][:, :, rt]                       # (out, I_PER, 2048)
        wcub[:2048] = blk.transpose(2, 1, 0).reshape(2048, M)
        w2blk = w2[:, I, :]                                 # (out, I_PER, 64)
        wcub[2048:2048 + D] = w2blk.transpose(2, 1, 0).reshape(D, M)
        d32blk = W_cu[:, I, :][:, :, d32_t] / 2
        wcub[2048 + D:] = d32blk.transpose(2, 1, 0).reshape(D, M)
        wcub = BT @ wcub                                    # sum-square fold
        wcubs.append(np.ascontiguousarray(wcub))

        wl = np.zeros((65, out), np.float32)
        if core == 0:
            wl[:D] = W_lin.T
            wl[D] = b
        wlins.append(wl)
    return wcubs, wlins


def _sel_consts():
    """Selection-SUM matrices, concatenated (64, 17*128).

    slot k in 0..15: col p has +1 at row a=(p%64) and +1 at row
    b=(a + 2k + p//64)%64 (coincident at d=0 -> value 2).
    slot 16: first 64 cols build the d32 sums (+1 at a, +1 at (a+32)%64).
    """
    sel = np.zeros((D, 17 * 128), np.float32)
    for k in range(NKCHUNK):
        for p in range(128):
            d = 2 * k + p // 64
            a = p % 64
            sel[a, k * 128 + p] += 1.0
            sel[(a + d) % D, k * 128 + p] += 1.0
    for a in range(D):
        sel[a, NKCHUNK * 128 + a] += 1.0
        sel[(a + 32) % D, NKCHUNK * 128 + a] += 1.0
    return sel


# ------------------------------------------------------------------ builder --

def _build_module():
    import concourse.bacc as bacc
    import concourse.mybir as mybir
    import concourse.tile as tile

    F32 = mybir.dt.float32
    F32R = mybir.dt.float32r
    MULT = mybir.AluOpType.mult
    ADD = mybir.AluOpType.add
    SQUARE = mybir.ActivationFunctionType.Square

    nc = bacc.Bacc("TRN2", target_bir_lowering=False, num_devices=NCORES, debug=False)

    x_in = nc.dram_tensor("x", [B, D], F32, kind="ExternalInput")
    wcub_in = [
        nc.dram_tensor(f"wcub{li}", [17 * 128, I_PER * OUTS[li]], F32, kind="ExternalInput")
        for li in range(3)
    ]
    wlin_in = [
        nc.dram_tensor(f"wlin{li}", [65, OUTS[li]], F32, kind="ExternalInput")
        for li in range(3)
    ]
    colsel_in = nc.dram_tensor("colsel", [D, I_PER], F32, kind="ExternalInput")
    out_ext = nc.dram_tensor("out", [B, OUTS[2]], F32, kind="ExternalOutput")

    sel_c = nc.inline_tensor(_sel_consts(), name="selc")
    ident_c = nc.inline_tensor(np.eye(128, dtype=np.float32), name="identc")

    with tile.TileContext(nc) as tc:
        with (
            tc.tile_pool(name="wpool", bufs=2) as wpool,
            tc.tile_pool(name="spool", bufs=1) as spool,
            tc.tile_pool(name="xpool", bufs=2) as xpool,
            tc.tile_pool(name="qpool", bufs=1) as qpool,
            tc.tile_pool(name="ypool", bufs=2) as ypool,
            tc.tile_pool(name="hpool", bufs=3) as hpool,
            tc.tile_pool(name="ps_rep", bufs=2, space="PSUM") as ps_rep,
            tc.tile_pool(name="ps_h", bufs=3, space="PSUM") as ps_h,
            tc.tile_pool(name="ps_small", bufs=3, space="PSUM") as ps_small,
            tc.tile_pool(name="dpool", bufs=2, space="DRAM") as dpool,
        ):
            # ---- warm-up collective: absorb ncfw init + cross-core skew
            warm_src = dpool.tile([128, 4], F32, tag="warm_src")
            warm_dst = dpool.tile([128, 4], F32, tag="warm_dst")
            warm_sb = spool.tile([128, 4], F32, tag="warm_sb")
            nc.vector.memset(warm_sb[:], 0.0)
            nc.sync.dma_start(warm_src[:], warm_sb[:])
            nc.gpsimd.collective_compute(
                "AllReduce",
                ADD,
                replica_groups=[list(range(NCORES))],
                ins=[warm_src.opt()],
                outs=[warm_dst.opt()],
            )

            sel_sb = spool.tile([D, 17 * 128], F32R, tag="sel")
            nc.sync.dma_start(sel_sb[:], sel_c.ap().bitcast(F32R))
            ident_sb = spool.tile([128, 128], F32, tag="ident")
            nc.sync.dma_start(ident_sb[:], ident_c.ap())
            colsel_sb = spool.tile([D, I_PER], F32R, tag="colsel")
            nc.sync.dma_start(colsel_sb[:], colsel_in.ap().bitcast(F32R))

            HB = 512            # half-batch
            NBH = HB // 128     # 4 chunks per half

            # per-layer weight tiles (split into two k-halves so the first
            # matmuls can start before the whole layer's weights land)
            weights = []
            for li in range(3):
                M = I_PER * OUTS[li]
                wcub_sb = wpool.tile([128, NKCHUNK, M], F32R, tag="wcub")
                for kh in range(2):
                    nc.sync.dma_start(
                        wcub_sb[:, kh * 8:(kh + 1) * 8, :],
                        wcub_in[li].ap().bitcast(F32R)[kh * 1024:(kh + 1) * 1024, :]
                        .rearrange("(k p) m -> p k m", p=128),
                    )
                wx_sb = wpool.tile([D, M], F32R, tag="wx")
                nc.sync.dma_start(wx_sb[:], wcub_in[li].ap().bitcast(F32R)[2048:2048 + D, :])
                wd32_sb = wpool.tile([D, M], F32R, tag="wd32")
                nc.sync.dma_start(wd32_sb[:], wcub_in[li].ap().bitcast(F32R)[2048 + D:, :])
                wlin_sb = wpool.tile([65, OUTS[li]], F32R, tag="wlin")
                nc.sync.dma_start(wlin_sb[:], wlin_in[li].ap().bitcast(F32R))
                weights.append((wcub_sb, wx_sb, wd32_sb, wlin_sb))

            # x tiles for layer 0, both halves, straight from the input
            x_half = []
            for h in range(2):
                xs = xpool.tile([128, NBH, D], F32, tag=f"x{h}")
                nc.sync.dma_start(
                    xs[:],
                    x_in.ap()[h * HB:(h + 1) * HB, :]
                    .rearrange("(bc p) f -> p bc f", p=128),
                )
                x_half.append(xs)

            for li in range(3):
                out_l = OUTS[li]
                M = I_PER * out_l
                last = li == 2
                wcub_sb, wx_sb, wd32_sb, wlin_sb = weights[li]
                next_x = [None, None]

                for h in range(2):
                    x_sb = x_half[h]

                    # -- phase A: xT (transposed x, bf16-free: stays f32r)
                    xT_sb = xpool.tile([65, HB], F32R, tag=f"xT{h}")
                    for bc in range(NBH):
                        xTp = ps_small.tile([D, 128], F32, tag="small")
                        nc.tensor.transpose(xTp[:], x_sb[:, bc, :], ident_sb[:])
                        nc.scalar.copy(xT_sb[0:D, bc * 128:(bc + 1) * 128], xTp[:])
                    nc.vector.memset(xT_sb[D:65, :].bitcast(F32), 1.0)

                    # d32 rows: u = (x_a + x_{a+32})^2 / 2
                    xd32_sb = xpool.tile([D, HB], F32R, tag=f"xd32{h}")
                    rep32 = ps_rep.tile([128, HB], F32, tag="rep")
                    nc.tensor.matmul(
                        rep32[0:D, :], sel_sb[:, NKCHUNK * 128:NKCHUNK * 128 + D],
                        xT_sb[0:D, :], start=True, stop=True,
                    )
                    nc.scalar.activation(
                        xd32_sb[:], rep32[0:D, :], SQUARE, scale=INV_SQRT2
                    )

                    # -- phase B: u chunks via selection-sum matmul + Square
                    xsq = []
                    for k in range(NKCHUNK):
                        rep = ps_rep.tile([128, HB], F32, tag="rep")
                        nc.tensor.matmul(
                            rep[:], sel_sb[:, k * 128:(k + 1) * 128],
                            xT_sb[0:D, :], start=True, stop=True,
                        )
                        xq = qpool.tile([128, HB], F32R, tag=f"xsq{k}h{h}")
                        nc.scalar.activation(
                            xq[:], rep[:], SQUARE, scale=INV_SQRT2
                        )
                        xsq.append(xq)

                    # -- phase C
                    y_sb = ypool.tile([128, NBH, out_l], F32, tag=f"y{h}")
                    if not last:
                        for bc in range(NBH):
                            bs = slice(bc * 128, (bc + 1) * 128)
                            h_ps = ps_h.tile([128, M], F32, tag="h")
                            for k in range(NKCHUNK):
                                nc.tensor.matmul(
                                    h_ps[:], xsq[k][:, bs], wcub_sb[:, k, :],
                                    start=(k == 0), stop=False,
                                )
                            nc.tensor.matmul(h_ps[:], xT_sb[0:D, bs], wx_sb[:], start=False, stop=False)
                            nc.tensor.matmul(h_ps[:], xd32_sb[:, bs], wd32_sb[:], start=False, stop=True)
                            h_sb = hpool.tile([128, M], F32, tag="h_sb")
                            nc.vector.tensor_copy(h_sb[:], h_ps[:])

                            lin_ps = ps_small.tile([128, out_l], F32, tag="small")
                            nc.tensor.matmul(lin_ps[:], xT_sb[0:65, bs], wlin_sb[:], start=True, stop=True)
                            xmac_ps = ps_small.tile([128, I_PER], F32, tag="small")
                            nc.tensor.matmul(xmac_ps[:], xT_sb[0:D, bs], colsel_sb[:], start=True, stop=True)
                            xmac_sb = ypool.tile([128, I_PER], F32, tag="xmac")
                            nc.scalar.copy(xmac_sb[:], xmac_ps[:])

                            nc.scalar.copy(y_sb[:, bc, :], lin_ps[:])
                            for il in range(I_PER):
                                nc.vector.scalar_tensor_tensor(
                                    y_sb[:, bc, :],
                                    h_sb[:, il * out_l:(il + 1) * out_l],
                                    xmac_sb[:, il:il + 1],
                                    y_sb[:, bc, :],
                                    op0=MULT, op1=ADD,
                                )

                        # -- phase D: AllReduce this half
                        y_bounce = dpool.tile([HB, out_l], F32, tag=f"ybounce{h}")
                        y_red = dpool.tile([HB, out_l], F32, tag=f"yred{h}")
                        nc.sync.dma_start(
                            y_bounce[:].rearrange("(bc p) o -> p bc o", p=128), y_sb[:]
                        )
                        nc.gpsimd.collective_compute(
                            "AllReduce",
                            ADD,
                            replica_groups=[list(range(NCORES))],
                            ins=[y_bounce.opt()],
                            outs=[y_red.opt()],
                        )
                        xs = xpool.tile([128, NBH, D], F32, tag=f"x{h}")
                        nc.sync.dma_start(
                            xs[:], y_red[:].rearrange("(bc p) f -> p bc f", p=128)
                        )
                        next_x[h] = xs
                    else:
                        # layer 2: stationary-W GEMM, transpose, MAC
                        h_ps = ps_h.tile([M, HB], F32, tag="h")
                        for k in range(NKCHUNK):
                            nc.tensor.matmul(
                                h_ps[:], wcub_sb[:, k, :], xsq[k][:],
                                start=(k == 0), stop=False,
                            )
                        nc.tensor.matmul(h_ps[:], wx_sb[:], xT_sb[0:D, :], start=False, stop=False)
                        nc.tensor.matmul(h_ps[:], wd32_sb[:], xd32_sb[:], start=False, stop=True)
                        h2_sb = ypool.tile([M, HB], F32, tag=f"h2{h}")
                        nc.vector.tensor_copy(h2_sb[:], h_ps[:])

                        for bc in range(NBH):
                            bs = slice(bc * 128, (bc + 1) * 128)
                            h2t_ps = ps_small.tile([128, M], F32, tag="small")
                            nc.tensor.transpose(h2t_ps[:], h2_sb[:, bs], ident_sb[0:M, 0:M])
                            h2t_sb = hpool.tile([128, M], F32, tag="h2t_sb")
                            nc.vector.tensor_copy(h2t_sb[:], h2t_ps[:])

                            lin_ps = ps_small.tile([128, out_l], F32, tag="small")
                            nc.tensor.matmul(lin_ps[:], xT_sb[0:65, bs], wlin_sb[:], start=True, stop=True)
                            xmac_ps = ps_small.tile([128, I_PER], F32, tag="small")
                            nc.tensor.matmul(xmac_ps[:], xT_sb[0:D, bs], colsel_sb[:], start=True, stop=True)
                            xmac_sb = ypool.tile([128, I_PER], F32, tag="xmac")
                            nc.scalar.copy(xmac_sb[:], xmac_ps[:])

                            nc.scalar.copy(y_sb[:, bc, :], lin_ps[:])
                            for il in range(I_PER):
                                nc.vector.scalar_tensor_tensor(
                                    y_sb[:, bc, :],
                                    h2t_sb[:, il * out_l:(il + 1) * out_l],
                                    xmac_sb[:, il:il + 1],
                                    y_sb[:, bc, :],
                                    op0=MULT, op1=ADD,
                                )

                        nc.sync.dma_start(
                            out_ext.ap()[h * HB:(h + 1) * HB, :]
                            .rearrange("(bc p) o -> p bc o", p=128),
                            y_sb[:],
                        )

                if not last:
                    x_half = next_x

    nc.compile()
    return nc


# ------------------------------------------------------------------- runner --

def kernel(x, W0, b0, W1, b1, W2, b2):
    from concourse.bass_utils import run_bass_kernel_spmd

    if "nc" not in _CACHE:
        _CACHE["nc"] = _build_module()
    nc = _CACHE["nc"]

    x = np.ascontiguousarray(np.asarray(x, np.float32))
    Ws = [np.asarray(W, np.float32) for W in (W0, W1, W2)]
    bs = [np.asarray(b_, np.float32) for b_ in (b0, b1, b2)]

    BT = _u_transform()
    wcubs, wlins = {}, {}
    for li in range(3):
        wcubs[li], wlins[li] = _prep_layer(Ws[li], bs[li], OUTS[li], BT)

    in_maps = []
    for core in range(NCORES):
        I = np.arange(core * I_PER, (core + 1) * I_PER)
        colsel = np.zeros((D, I_PER), np.float32)
        colsel[I, np.arange(I_PER)] = 1.0
        m = {"x": x, "colsel": colsel}
        for li in range(3):
            m[f"wcub{li}"] = wcubs[li][core]
            m[f"wlin{li}"] = wlins[li][core]
        in_maps.append(m)

    res = run_bass_kernel_spmd(nc, in_maps, core_ids=list(range(NCORES)))
    out = np.zeros((B, OUTS[2]), np.float32)
    for core in range(NCORES):
        out += res.results[core]["out"]
    return out


# revision 9
# speedup vs baseline: 1.4725x; 1.2095x over previous
"""Trainium2 Bass kernel for nn_CubicModelLarge (3-layer cubic-feature MLP).

Strategy: tensor-parallel over the cubic multiplier index i (64 values, 8 per
core).  The cubic expansion is never materialized.  Per layer:

  y[b,o] = W_lin@x + b + sum_t W_sq[o,t] xsq[b,t] + sum_i x[b,i] sum_t W_cu[o,i,t] xsq[b,t]

Rewritten per core c (i in I_c = [8c, 8c+8)):

  H[b,(il,o)] = sum_J F[J,b] * Wcub[J,(il,o)]     (one f32r GEMM, J = 2176 rows)
  y_c[b,o]    = lin[b,o] + sum_il xmac[b,il] * H[b,(il,o)]
  y = AllReduce_c(y_c)

F rows use the sum-square basis: instead of products x_a*x_b, each row is
u = (x_a+x_b)^2/2, built by a selection-SUM matmul on the PE (two 1s per
column) followed by a Square activation on the Scalar engine (PSUM->SBUF).
This removes all DVE tensor_mul product work.  The weight fold
x_a*x_b = u_ab - x_a^2/2 - x_b^2/2 is applied host-side (corrections land on
the d=0 rows, whose value is now 2*x_a^2).

A tiny warm-up AllReduce issues first to absorb collective-init latency.
Final layer partials are summed on the host.
"""

import numpy as np

D = 64
B = 1024
NCORES = 8
I_PER = D // NCORES          # 8
OUTS = (64, 64, 10)
NKCHUNK = 16                 # rotation chunks (d pairs)
NB = B // 128                # 8 batch chunks
INV_SQRT2 = 0.7071067811865476

_CACHE = {}


# ---------------------------------------------------------------- host prep --

def _maps():
    iu, ju = np.triu_indices(D)
    tmap = np.zeros((D, D), np.int64)
    tmap[iu, ju] = np.arange(len(iu))
    tmap[ju, iu] = tmap[iu, ju]
    p = np.arange(128)
    rows_t = np.zeros((NKCHUNK, 128), np.int64)
    for k in range(NKCHUNK):
        d = 2 * k + p // 64
        a = p % 64
        rows_t[k] = tmap[a, (a + d) % D]
    d32_t = tmap[np.arange(D), (np.arange(D) + 32) % D]
    return tmap, rows_t, d32_t


def _u_transform():
    """B.T for the sum-square basis change on the 2176-row F basis.

    Rows 0..2047: rotation products (k = r//128, p = r%128, d = 2k + p//64,
    a = p%64, b = (a+d)%64).  Rows 2048..2111: x rows.  Rows 2112..2175:
    d32 products.  Old row value x_a*x_b = u_r - u_{d0(a)}/4 - u_{d0(b)}/4
    (a != b); d0 rows (a == b): x_a^2 = u_r/2.  d0(a) = row a.
    """
    n = 17 * 128
    Bm = np.zeros((n, n), np.float32)
    for r in range(2048):
        k, p = divmod(r, 128)
        d = 2 * k + p // 64
        a = p % 64
        b = (a + d) % D
        if a == b:
            Bm[r, r] = 0.5
        else:
            Bm[r, r] = 1.0
            Bm[r, a] -= 0.25
            Bm[r, b] -= 0.25
    for r in range(2048, 2048 + D):
        Bm[r, r] = 1.0
    for r in range(2048 + D, n):
        a = r - (2048 + D)
        b = (a + 32) % D
        Bm[r, r] = 1.0
        Bm[r, a] -= 0.25
        Bm[r, b] -= 0.25
    return Bm.T.copy()


def _prep_layer(W, b, out, BT):
    """-> (wcub [NCORES](2176, I_PER*out), wlin [NCORES](65, out))"""
    _, rows_t, d32_t = _maps()
    W_lin = W[:, :D]
    W_sq = W[:, D:D + 2080]
    W_cu = W[:, D + 2080:].reshape(out, D, 2080)

    iu, ju = np.triu_indices(D)
    w2 = np.zeros((out, D, D), np.float32)
    half = np.where(iu == ju, 1.0, 0.5).astype(np.float32)
    w2[:, iu, ju] = W_sq * half
    w2[:, ju, iu] = W_sq * half

    rt = rows_t.reshape(-1)
    wcubs, wlins = [], []
    for core in range(NCORES):
        I = np.arange(core * I_PER, (core + 1) * I_PER)
        M = I_PER * out
        wcub = np.zeros((17 * 128, M), np.float32)
        blk = W_cu[:, I, :][:, :, rt]                       # (out, I_PER, 2048)
        wcub[:2048] = blk.transpose(2, 1, 0).reshape(2048, M)
        w2blk = w2[:, I, :]                                 # (out, I_PER, 64)
        wcub[2048:2048 + D] = w2blk.transpose(2, 1, 0).reshape(D, M)
        d32blk = W_cu[:, I, :][:, :, d32_t] / 2
        wcub[2048 + D:] = d32blk.transpose(2, 1, 0).reshape(D, M)
        wcub = BT @ wcub                                    # sum-square fold
        wcubs.append(np.ascontiguousarray(wcub.astype(np.float16)))

        wl = np.zeros((65, out), np.float32)
        if core == 0:
            wl[:D] = W_lin.T
            wl[D] = b
        wlins.append(wl.astype(np.float16))
    return wcubs, wlins


def _sel_consts():
    """Selection-SUM matrices, concatenated (64, 17*128).

    slot k in 0..15: col p has +1 at row a=(p%64) and +1 at row
    b=(a + 2k + p//64)%64 (coincident at d=0 -> value 2).
    slot 16: first 64 cols build the d32 sums (+1 at a, +1 at (a+32)%64).
    """
    sel = np.zeros((D, 17 * 128), np.float16)
    for k in range(NKCHUNK):
        for p in range(128):
            d = 2 * k + p // 64
            a = p % 64
            sel[a, k * 128 + p] += 1.0
            sel[(a + d) % D, k * 128 + p] += 1.0
    for a in range(D):
        sel[a, NKCHUNK * 128 + a] += 1.0
        sel[(a + 32) % D, NKCHUNK * 128 + a] += 1.0
    return sel


# ------------------------------------------------------------------ builder --

def _build_module():
    import concourse.bacc as bacc
    import concourse.mybir as mybir
    import concourse.tile as tile

    F32 = mybir.dt.float32
    F32R = mybir.dt.float32r
    F16 = mybir.dt.float16
    MULT = mybir.AluOpType.mult
    ADD = mybir.AluOpType.add
    SQUARE = mybir.ActivationFunctionType.Square

    nc = bacc.Bacc("TRN2", target_bir_lowering=False, num_devices=NCORES, debug=False)

    x_in = nc.dram_tensor("x", [B, D], F32, kind="ExternalInput")
    wcub_in = [
        nc.dram_tensor(f"wcub{li}", [17 * 128, I_PER * OUTS[li]], F16, kind="ExternalInput")
        for li in range(3)
    ]
    wlin_in = [
        nc.dram_tensor(f"wlin{li}", [65, OUTS[li]], F16, kind="ExternalInput")
        for li in range(3)
    ]
    colsel_in = nc.dram_tensor("colsel", [D, I_PER], F16, kind="ExternalInput")
    out_ext = nc.dram_tensor("out", [B, OUTS[2]], F32, kind="ExternalOutput")

    sel_c = nc.inline_tensor(_sel_consts(), name="selc")
    ident_c = nc.inline_tensor(np.eye(128, dtype=np.float32), name="identc")

    with tile.TileContext(nc) as tc:
        with (
            tc.tile_pool(name="wpool", bufs=2) as wpool,
            tc.tile_pool(name="spool", bufs=1) as spool,
            tc.tile_pool(name="xpool", bufs=2) as xpool,
            tc.tile_pool(name="qpool", bufs=1) as qpool,
            tc.tile_pool(name="ypool", bufs=2) as ypool,
            tc.tile_pool(name="hpool", bufs=3) as hpool,
            tc.tile_pool(name="ps_rep", bufs=2, space="PSUM") as ps_rep,
            tc.tile_pool(name="ps_h", bufs=3, space="PSUM") as ps_h,
            tc.tile_pool(name="ps_small", bufs=3, space="PSUM") as ps_small,
            tc.tile_pool(name="dpool", bufs=2, space="DRAM") as dpool,
        ):
            # ---- warm-up collective: absorb ncfw init + cross-core skew
            warm_src = dpool.tile([128, 4], F32, tag="warm_src")
            warm_dst = dpool.tile([128, 4], F32, tag="warm_dst")
            warm_sb = spool.tile([128, 4], F32, tag="warm_sb")
            nc.vector.memset(warm_sb[:], 0.0)
            nc.sync.dma_start(warm_src[:], warm_sb[:])
            nc.gpsimd.collective_compute(
                "AllReduce",
                ADD,
                replica_groups=[list(range(NCORES))],
                ins=[warm_src.opt()],
                outs=[warm_dst.opt()],
            )

            sel_sb = spool.tile([D, 17 * 128], F16, tag="sel")
            nc.sync.dma_start(sel_sb[:], sel_c.ap())
            ident_sb = spool.tile([128, 128], F32, tag="ident")
            nc.sync.dma_start(ident_sb[:], ident_c.ap())
            colsel_sb = spool.tile([D, I_PER], F16, tag="colsel")
            nc.sync.dma_start(colsel_sb[:], colsel_in.ap())

            HB = 512            # half-batch
            NBH = HB // 128     # 4 chunks per half

            # per-layer weight tiles (split into two k-halves so the first
            # matmuls can start before the whole layer's weights land)
            weights = []
            for li in range(3):
                M = I_PER * OUTS[li]
                wcub_sb = wpool.tile([128, NKCHUNK, M], F16, tag="wcub")
                for kh in range(2):
                    nc.sync.dma_start(
                        wcub_sb[:, kh * 8:(kh + 1) * 8, :],
                        wcub_in[li].ap()[kh * 1024:(kh + 1) * 1024, :]
                        .rearrange("(k p) m -> p k m", p=128),
                    )
                wx_sb = wpool.tile([D, M], F16, tag="wx")
                nc.sync.dma_start(wx_sb[:], wcub_in[li].ap()[2048:2048 + D, :])
                wd32_sb = wpool.tile([D, M], F16, tag="wd32")
                nc.sync.dma_start(wd32_sb[:], wcub_in[li].ap()[2048 + D:, :])
                wlin_sb = wpool.tile([65, OUTS[li]], F16, tag="wlin")
                nc.sync.dma_start(wlin_sb[:], wlin_in[li].ap())
                weights.append((wcub_sb, wx_sb, wd32_sb, wlin_sb))

            # x tiles for layer 0, both halves, straight from the input
            x_half = []
            for h in range(2):
                xs = xpool.tile([128, NBH, D], F32, tag=f"x{h}")
                nc.sync.dma_start(
                    xs[:],
                    x_in.ap()[h * HB:(h + 1) * HB, :]
                    .rearrange("(bc p) f -> p bc f", p=128),
                )
                x_half.append(xs)

            for li in range(3):
                out_l = OUTS[li]
                M = I_PER * out_l
                last = li == 2
                wcub_sb, wx_sb, wd32_sb, wlin_sb = weights[li]
                next_x = [None, None]

                for h in range(2):
                    x_sb = x_half[h]

                    # -- phase A: xT (transposed x, bf16-free: stays f32r)
                    xT_sb = xpool.tile([65, HB], F16, tag=f"xT{h}")
                    for bc in range(NBH):
                        xTp = ps_small.tile([D, 128], F32, tag="small")
                        nc.tensor.transpose(xTp[:], x_sb[:, bc, :], ident_sb[:])
                        nc.scalar.copy(xT_sb[0:D, bc * 128:(bc + 1) * 128], xTp[:])
                    nc.vector.memset(xT_sb[D:65, :], 1.0)

                    # d32 rows: u = (x_a + x_{a+32})^2 / 2
                    xd32_sb = xpool.tile([D, HB], F16, tag=f"xd32{h}")
                    rep32 = ps_rep.tile([128, HB], F32, tag="rep")
                    nc.tensor.matmul(
                        rep32[0:D, :], sel_sb[:, NKCHUNK * 128:NKCHUNK * 128 + D],
                        xT_sb[0:D, :], start=True, stop=True,
                    )
                    nc.scalar.activation(
                        xd32_sb[:], rep32[0:D, :], SQUARE, scale=INV_SQRT2
                    )

                    # -- phase B: u chunks via selection-sum matmul + Square
                    xsq = []
                    for k in range(NKCHUNK):
                        rep = ps_rep.tile([128, HB], F32, tag="rep")
                        nc.tensor.matmul(
                            rep[:], sel_sb[:, k * 128:(k + 1) * 128],
                            xT_sb[0:D, :], start=True, stop=True,
                        )
                        xq = qpool.tile([128, HB], F16, tag=f"xsq{k}h{h}")
                        nc.scalar.activation(
                            xq[:], rep[:], SQUARE, scale=INV_SQRT2
                        )
                        xsq.append(xq)

                    # -- phase C
                    y_sb = ypool.tile([128, NBH, out_l], F32, tag=f"y{h}")
                    if not last:
                        for bc in range(NBH):
                            bs = slice(bc * 128, (bc + 1) * 128)
                            h_ps = ps_h.tile([128, M], F32, tag="h")
                            for k in range(NKCHUNK):
                                nc.tensor.matmul(
                                    h_ps[:], xsq[k][:, bs], wcub_sb[:, k, :],
                                    start=(k == 0), stop=False,
                                )
                            nc.tensor.matmul(h_ps[:], xT_sb[0:D, bs], wx_sb[:], start=False, stop=False)
                            nc.tensor.matmul(h_ps[:], xd32_sb[:, bs], wd32_sb[:], start=False, stop=True)
                            h_sb = hpool.tile([128, M], F32, tag="h_sb")
                            nc.vector.tensor_copy(h_sb[:], h_ps[:])

                            lin_ps = ps_small.tile([128, out_l], F32, tag="small")
                            nc.tensor.matmul(lin_ps[:], xT_sb[0:65, bs], wlin_sb[:], start=True, stop=True)
                            xmac_ps = ps_small.tile([128, I_PER], F32, tag="small")
                            nc.tensor.matmul(xmac_ps[:], xT_sb[0:D, bs], colsel_sb[:], start=True, stop=True)
                            xmac_sb = ypool.tile([128, I_PER], F32, tag="xmac")
                            nc.scalar.copy(xmac_sb[:], xmac_ps[:])

                            nc.scalar.copy(y_sb[:, bc, :], lin_ps[:])
                            for il in range(I_PER):
                                nc.vector.scalar_tensor_tensor(
                                    y_sb[:, bc, :],
                                    h_sb[:, il * out_l:(il + 1) * out_l],
                                    xmac_sb[:, il:il + 1],
                                    y_sb[:, bc, :],
                                    op0=MULT, op1=ADD,
                                )

                        # -- phase D: AllReduce this half
                        y_bounce = dpool.tile([HB, out_l], F32, tag=f"ybounce{h}")
                        y_red = dpool.tile([HB, out_l], F32, tag=f"yred{h}")
                        nc.sync.dma_start(
                            y_bounce[:].rearrange("(bc p) o -> p bc o", p=128), y_sb[:]
                        )
                        nc.gpsimd.collective_compute(
                            "AllReduce",
                            ADD,
                            replica_groups=[list(range(NCORES))],
                            ins=[y_bounce.opt()],
                            outs=[y_red.opt()],
                        )
                        xs = xpool.tile([128, NBH, D], F32, tag=f"x{h}")
                        nc.sync.dma_start(
                            xs[:], y_red[:].rearrange("(bc p) f -> p bc f", p=128)
                        )
                        next_x[h] = xs
                    else:
                        # layer 2: stationary-W GEMM, transpose, MAC
                        h_ps = ps_h.tile([M, HB], F32, tag="h")
                        for k in range(NKCHUNK):
                            nc.tensor.matmul(
                                h_ps[:], wcub_sb[:, k, :], xsq[k][:],
                                start=(k == 0), stop=False,
                            )
                        nc.tensor.matmul(h_ps[:], wx_sb[:], xT_sb[0:D, :], start=False, stop=False)
                        nc.tensor.matmul(h_ps[:], wd32_sb[:], xd32_sb[:], start=False, stop=True)
                        h2_sb = ypool.tile([M, HB], F32, tag=f"h2{h}")
                        nc.vector.tensor_copy(h2_sb[:], h_ps[:])

                        for bc in range(NBH):
                            bs = slice(bc * 128, (bc + 1) * 128)
                            h2t_ps = ps_small.tile([128, M], F32, tag="small")
                            nc.tensor.transpose(h2t_ps[:], h2_sb[:, bs], ident_sb[0:M, 0:M])
                            h2t_sb = hpool.tile([128, M], F32, tag="h2t_sb")
                            nc.vector.tensor_copy(h2t_sb[:], h2t_ps[:])

                            lin_ps = ps_small.tile([128, out_l], F32, tag="small")
                            nc.tensor.matmul(lin_ps[:], xT_sb[0:65, bs], wlin_sb[:], start=True, stop=True)
                            xmac_ps = ps_small.tile([128, I_PER], F32, tag="small")
                            nc.tensor.matmul(xmac_ps[:], xT_sb[0:D, bs], colsel_sb[:], start=True, stop=True)
                            xmac_sb = ypool.tile([128, I_PER], F32, tag="xmac")
                            nc.scalar.copy(xmac_sb[:], xmac_ps[:])

                            nc.scalar.copy(y_sb[:, bc, :], lin_ps[:])
                            for il in range(I_PER):
                                nc.vector.scalar_tensor_tensor(
                                    y_sb[:, bc, :],
                                    h2t_sb[:, il * out_l:(il + 1) * out_l],
                                    xmac_sb[:, il:il + 1],
                                    y_sb[:, bc, :],
                                    op0=MULT, op1=ADD,
                                )

                        nc.sync.dma_start(
                            out_ext.ap()[h * HB:(h + 1) * HB, :]
                            .rearrange("(bc p) o -> p bc o", p=128),
                            y_sb[:],
                        )

                if not last:
                    x_half = next_x

    nc.compile()
    return nc


# ------------------------------------------------------------------- runner --

def kernel(x, W0, b0, W1, b1, W2, b2):
    from concourse.bass_utils import run_bass_kernel_spmd

    if "nc" not in _CACHE:
        _CACHE["nc"] = _build_module()
    nc = _CACHE["nc"]

    x = np.ascontiguousarray(np.asarray(x, np.float32))
    Ws = [np.asarray(W, np.float32) for W in (W0, W1, W2)]
    bs = [np.asarray(b_, np.float32) for b_ in (b0, b1, b2)]

    BT = _u_transform()
    wcubs, wlins = {}, {}
    for li in range(3):
        wcubs[li], wlins[li] = _prep_layer(Ws[li], bs[li], OUTS[li], BT)

    in_maps = []
    for core in range(NCORES):
        I = np.arange(core * I_PER, (core + 1) * I_PER)
        colsel = np.zeros((D, I_PER), np.float16)
        colsel[I, np.arange(I_PER)] = 1.0
        m = {"x": x, "colsel": colsel}
        for li in range(3):
            m[f"wcub{li}"] = wcubs[li][core]
            m[f"wlin{li}"] = wlins[li][core]
        in_maps.append(m)

    res = run_bass_kernel_spmd(nc, in_maps, core_ids=list(range(NCORES)))
    out = np.zeros((B, OUTS[2]), np.float32)
    for core in range(NCORES):
        out += res.results[core]["out"]
    return out


# revision 13
# speedup vs baseline: 1.5018x; 1.0199x over previous
"""Trainium2 Bass kernel for nn_CubicModelLarge (3-layer cubic-feature MLP).

Strategy: tensor-parallel over the cubic multiplier index i (64 values, 8 per
core).  The cubic expansion is never materialized.  Per layer:

  y[b,o] = W_lin@x + b + sum_t W_sq[o,t] xsq[b,t] + sum_i x[b,i] sum_t W_cu[o,i,t] xsq[b,t]

Rewritten per core c (i in I_c = [8c, 8c+8)):

  H[b,(il,o)] = sum_J F[J,b] * Wcub[J,(il,o)]     (one f32r GEMM, J = 2176 rows)
  y_c[b,o]    = lin[b,o] + sum_il xmac[b,il] * H[b,(il,o)]
  y = AllReduce_c(y_c)

F rows use the sum-square basis: instead of products x_a*x_b, each row is
u = (x_a+x_b)^2/2, built by a selection-SUM matmul on the PE (two 1s per
column) followed by a Square activation on the Scalar engine (PSUM->SBUF).
This removes all DVE tensor_mul product work.  The weight fold
x_a*x_b = u_ab - x_a^2/2 - x_b^2/2 is applied host-side (corrections land on
the d=0 rows, whose value is now 2*x_a^2).

A tiny warm-up AllReduce issues first to absorb collective-init latency.
Final layer partials are summed on the host.
"""

import numpy as np

D = 64
B = 1024
NCORES = 8
I_PER = D // NCORES          # 8
OUTS = (64, 64, 10)
NKCHUNK = 16                 # rotation chunks (d pairs)
NB = B // 128                # 8 batch chunks
INV_SQRT2 = 0.7071067811865476

_CACHE = {}


# ---------------------------------------------------------------- host prep --

def _maps():
    iu, ju = np.triu_indices(D)
    tmap = np.zeros((D, D), np.int64)
    tmap[iu, ju] = np.arange(len(iu))
    tmap[ju, iu] = tmap[iu, ju]
    p = np.arange(128)
    rows_t = np.zeros((NKCHUNK, 128), np.int64)
    for k in range(NKCHUNK):
        d = 2 * k + p // 64
        a = p % 64
        rows_t[k] = tmap[a, (a + d) % D]
    d32_t = tmap[np.arange(D), (np.arange(D) + 32) % D]
    return tmap, rows_t, d32_t


def _u_transform():
    """B.T for the sum-square basis change on the 2176-row F basis.

    Rows 0..2047: rotation products (k = r//128, p = r%128, d = 2k + p//64,
    a = p%64, b = (a+d)%64).  Rows 2048..2111: x rows.  Rows 2112..2175:
    d32 products.  Old row value x_a*x_b = u_r - u_{d0(a)}/4 - u_{d0(b)}/4
    (a != b); d0 rows (a == b): x_a^2 = u_r/2.  d0(a) = row a.
    """
    n = 17 * 128
    Bm = np.zeros((n, n), np.float32)
    for r in range(2048):
        k, p = divmod(r, 128)
        d = 2 * k + p // 64
        a = p % 64
        b = (a + d) % D
        if a == b:
            Bm[r, r] = 0.5
        else:
            Bm[r, r] = 1.0
            Bm[r, a] -= 0.25
            Bm[r, b] -= 0.25
    for r in range(2048, 2048 + D):
        Bm[r, r] = 1.0
    for r in range(2048 + D, n):
        a = r - (2048 + D)
        b = (a + 32) % D
        Bm[r, r] = 1.0
        Bm[r, a] -= 0.25
        Bm[r, b] -= 0.25
    return Bm.T.copy()


def _prep_layer(W, b, out, BT):
    """-> (wcub [NCORES](2176, I_PER*out), wlin [NCORES](65, out))"""
    _, rows_t, d32_t = _maps()
    W_lin = W[:, :D]
    W_sq = W[:, D:D + 2080]
    W_cu = W[:, D + 2080:].reshape(out, D, 2080)

    iu, ju = np.triu_indices(D)
    w2 = np.zeros((out, D, D), np.float32)
    half = np.where(iu == ju, 1.0, 0.5).astype(np.float32)
    w2[:, iu, ju] = W_sq * half
    w2[:, ju, iu] = W_sq * half

    rt = rows_t.reshape(-1)
    wcubs, wlins = [], []
    for core in range(NCORES):
        I = np.arange(core * I_PER, (core + 1) * I_PER)
        M = I_PER * out
        wcub = np.zeros((17 * 128, M), np.float32)
        blk = W_cu[:, I, :][:, :, rt]                       # (out, I_PER, 2048)
        wcub[:2048] = blk.transpose(2, 1, 0).reshape(2048, M)
        w2blk = w2[:, I, :]                                 # (out, I_PER, 64)
        wcub[2048:2048 + D] = w2blk.transpose(2, 1, 0).reshape(D, M)
        d32blk = W_cu[:, I, :][:, :, d32_t] / 2
        wcub[2048 + D:] = d32blk.transpose(2, 1, 0).reshape(D, M)
        wcub = BT @ wcub                                    # sum-square fold
        wcubs.append(np.ascontiguousarray(wcub.astype(np.float16)))

        wl = np.zeros((65, out), np.float32)
        if core == 0:
            wl[:D] = W_lin.T
            wl[D] = b
        wlins.append(wl.astype(np.float16))
    return wcubs, wlins


def _sel_consts():
    """Selection-SUM matrices, concatenated (64, 17*128).

    slot k in 0..15: col p has +1 at row a=(p%64) and +1 at row
    b=(a + 2k + p//64)%64 (coincident at d=0 -> value 2).
    slot 16: first 64 cols build the d32 sums (+1 at a, +1 at (a+32)%64).
    """
    sel = np.zeros((D, 17 * 128), np.float16)
    for k in range(NKCHUNK):
        for p in range(128):
            d = 2 * k + p // 64
            a = p % 64
            sel[a, k * 128 + p] += 1.0
            sel[(a + d) % D, k * 128 + p] += 1.0
    for a in range(D):
        sel[a, NKCHUNK * 128 + a] += 1.0
        sel[(a + 32) % D, NKCHUNK * 128 + a] += 1.0
    return sel


# ------------------------------------------------------------------ builder --

def _build_module():
    import concourse.bacc as bacc
    import concourse.mybir as mybir
    import concourse.tile as tile

    F32 = mybir.dt.float32
    F32R = mybir.dt.float32r
    F16 = mybir.dt.float16
    MULT = mybir.AluOpType.mult
    ADD = mybir.AluOpType.add
    SQUARE = mybir.ActivationFunctionType.Square
    AXIS_X = mybir.AxisListType.X

    nc = bacc.Bacc("TRN2", target_bir_lowering=False, num_devices=NCORES, debug=False)

    x_in = nc.dram_tensor("x", [B, D], F32, kind="ExternalInput")
    wcub_in = [
        nc.dram_tensor(f"wcub{li}", [17 * 128, I_PER * OUTS[li]], F16, kind="ExternalInput")
        for li in range(3)
    ]
    wlin_in = [
        nc.dram_tensor(f"wlin{li}", [65, OUTS[li]], F16, kind="ExternalInput")
        for li in range(3)
    ]
    colsel_in = nc.dram_tensor("colsel", [D, I_PER], F16, kind="ExternalInput")
    out_ext = nc.dram_tensor("out", [B, OUTS[2]], F32, kind="ExternalOutput")

    sel_c = nc.inline_tensor(_sel_consts(), name="selc")
    ident_c = nc.inline_tensor(np.eye(128, dtype=np.float32), name="identc")

    with tile.TileContext(nc) as tc:
        with (
            tc.tile_pool(name="wpool", bufs=2) as wpool,
            tc.tile_pool(name="spool", bufs=1) as spool,
            tc.tile_pool(name="xpool", bufs=2) as xpool,
            tc.tile_pool(name="qpool", bufs=1) as qpool,
            tc.tile_pool(name="ypool", bufs=2) as ypool,
            tc.tile_pool(name="hpool", bufs=3) as hpool,
            tc.tile_pool(name="ps_rep", bufs=2, space="PSUM") as ps_rep,
            tc.tile_pool(name="ps_h", bufs=3, space="PSUM") as ps_h,
            tc.tile_pool(name="ps_small", bufs=3, space="PSUM") as ps_small,
            tc.tile_pool(name="dpool", bufs=2, space="DRAM") as dpool,
        ):
            # ---- warm-up collective: absorb ncfw init + cross-core skew
            warm_src = dpool.tile([128, 4], F32, tag="warm_src")
            warm_dst = dpool.tile([128, 4], F32, tag="warm_dst")
            warm_sb = spool.tile([128, 4], F32, tag="warm_sb")
            nc.vector.memset(warm_sb[:], 0.0)
            nc.sync.dma_start(warm_src[:], warm_sb[:])
            nc.gpsimd.collective_compute(
                "AllReduce",
                ADD,
                replica_groups=[list(range(NCORES))],
                ins=[warm_src.opt()],
                outs=[warm_dst.opt()],
            )

            sel_sb = spool.tile([D, 17 * 128], F16, tag="sel")
            nc.sync.dma_start(sel_sb[:], sel_c.ap())
            ident_sb = spool.tile([128, 128], F32, tag="ident")
            nc.sync.dma_start(ident_sb[:], ident_c.ap())
            colsel_sb = spool.tile([D, I_PER], F16, tag="colsel")
            nc.sync.dma_start(colsel_sb[:], colsel_in.ap())

            HB = 512            # half-batch
            NBH = HB // 128     # 4 chunks per half

            # per-layer weight tiles (split into two k-halves so the first
            # matmuls can start before the whole layer's weights land)
            weights = []
            for li in range(3):
                M = I_PER * OUTS[li]
                wcub_sb = wpool.tile([128, NKCHUNK, M], F16, tag="wcub")
                for kh in range(2):
                    nc.sync.dma_start(
                        wcub_sb[:, kh * 8:(kh + 1) * 8, :],
                        wcub_in[li].ap()[kh * 1024:(kh + 1) * 1024, :]
                        .rearrange("(k p) m -> p k m", p=128),
                    )
                wx_sb = wpool.tile([D, M], F16, tag="wx")
                nc.sync.dma_start(wx_sb[:], wcub_in[li].ap()[2048:2048 + D, :])
                wd32_sb = wpool.tile([D, M], F16, tag="wd32")
                nc.sync.dma_start(wd32_sb[:], wcub_in[li].ap()[2048 + D:, :])
                wlin_sb = wpool.tile([65, OUTS[li]], F16, tag="wlin")
                nc.sync.dma_start(wlin_sb[:], wlin_in[li].ap())
                weights.append((wcub_sb, wx_sb, wd32_sb, wlin_sb))

            # x tiles for layer 0, both halves, straight from the input
            x_half = []
            for h in range(2):
                xs = xpool.tile([128, NBH, D], F32, tag=f"x{h}")
                nc.sync.dma_start(
                    xs[:],
                    x_in.ap()[h * HB:(h + 1) * HB, :]
                    .rearrange("(bc p) f -> p bc f", p=128),
                )
                x_half.append(xs)

            for li in range(3):
                out_l = OUTS[li]
                M = I_PER * out_l
                last = li == 2
                wcub_sb, wx_sb, wd32_sb, wlin_sb = weights[li]
                next_x = [None, None]

                for h in range(2):
                    x_sb = x_half[h]

                    # -- phase A: xT (transposed x, bf16-free: stays f32r)
                    xT_sb = xpool.tile([65, HB], F16, tag=f"xT{h}")
                    for bc in range(NBH):
                        xTp = ps_small.tile([D, 128], F32, tag="small")
                        nc.tensor.transpose(xTp[:], x_sb[:, bc, :], ident_sb[:])
                        nc.scalar.copy(xT_sb[0:D, bc * 128:(bc + 1) * 128], xTp[:])
                    nc.vector.memset(xT_sb[D:65, :], 1.0)

                    # d32 rows: u = (x_a + x_{a+32})^2 / 2
                    xd32_sb = xpool.tile([D, HB], F16, tag=f"xd32{h}")
                    rep32 = ps_rep.tile([128, HB], F32, tag="rep")
                    nc.tensor.matmul(
                        rep32[0:D, :], sel_sb[:, NKCHUNK * 128:NKCHUNK * 128 + D],
                        xT_sb[0:D, :], start=True, stop=True,
                    )
                    nc.scalar.activation(
                        xd32_sb[:], rep32[0:D, :], SQUARE, scale=INV_SQRT2
                    )

                    # -- phase B: u chunks via selection-sum matmul + Square
                    xsq = []
                    for k in range(NKCHUNK):
                        rep = ps_rep.tile([128, HB], F32, tag="rep")
                        nc.tensor.matmul(
                            rep[:], sel_sb[:, k * 128:(k + 1) * 128],
                            xT_sb[0:D, :], start=True, stop=True,
                        )
                        xq = qpool.tile([128, HB], F16, tag=f"xsq{k}h{h}")
                        nc.scalar.activation(
                            xq[:], rep[:], SQUARE, scale=INV_SQRT2
                        )
                        xsq.append(xq)

                    # -- phase C
                    y_sb = ypool.tile([128, NBH, out_l], F32, tag=f"y{h}")
                    if not last:
                        for bc in range(NBH):
                            bs = slice(bc * 128, (bc + 1) * 128)
                            h_ps = ps_h.tile([128, M], F32, tag="h")
                            for k in range(NKCHUNK):
                                nc.tensor.matmul(
                                    h_ps[:], xsq[k][:, bs], wcub_sb[:, k, :],
                                    start=(k == 0), stop=False,
                                )
                            nc.tensor.matmul(h_ps[:], xT_sb[0:D, bs], wx_sb[:], start=False, stop=False)
                            nc.tensor.matmul(h_ps[:], xd32_sb[:, bs], wd32_sb[:], start=False, stop=True)

                            lin_ps = ps_small.tile([128, out_l], F32, tag="small")
                            nc.tensor.matmul(lin_ps[:], xT_sb[0:65, bs], wlin_sb[:], start=True, stop=True)
                            xmac_ps = ps_small.tile([128, I_PER], F32, tag="small")
                            nc.tensor.matmul(xmac_ps[:], xT_sb[0:D, bs], colsel_sb[:], start=True, stop=True)
                            xmac_sb = ypool.tile([128, I_PER], F32, tag="xmac")
                            nc.scalar.copy(xmac_sb[:], xmac_ps[:])

                            # tmp[:, :M] = h * xmac (broadcast over o); tmp[:, M:] = lin
                            tmp_sb = hpool.tile([128, M + out_l], F32, tag="tmp")
                            xmac_b = (
                                xmac_sb[:].unsqueeze(2).to_broadcast([128, I_PER, out_l])
                            )
                            nc.vector.tensor_tensor(
                                tmp_sb[:, 0:M].rearrange("p (i o) -> p i o", i=I_PER),
                                h_ps[:].rearrange("p (i o) -> p i o", i=I_PER),
                                xmac_b,
                                op=MULT,
                            )
                            nc.scalar.copy(tmp_sb[:, M:], lin_ps[:])
                            nc.vector.tensor_reduce(
                                y_sb[:, bc, :],
                                tmp_sb[:].rearrange("p (i o) -> p o i", i=I_PER + 1),
                                axis=AXIS_X, op=ADD,
                            )

                        # -- phase D: AllReduce this half
                        y_bounce = dpool.tile([HB, out_l], F32, tag=f"ybounce{h}")
                        y_red = dpool.tile([HB, out_l], F32, tag=f"yred{h}")
                        nc.sync.dma_start(
                            y_bounce[:].rearrange("(bc p) o -> p bc o", p=128), y_sb[:]
                        )
                        nc.gpsimd.collective_compute(
                            "AllReduce",
                            ADD,
                            replica_groups=[list(range(NCORES))],
                            ins=[y_bounce.opt()],
                            outs=[y_red.opt()],
                        )
                        xs = xpool.tile([128, NBH, D], F32, tag=f"x{h}")
                        nc.sync.dma_start(
                            xs[:], y_red[:].rearrange("(bc p) f -> p bc f", p=128)
                        )
                        next_x[h] = xs
                    else:
                        # layer 2: stationary-W GEMM, transpose, MAC
                        h_ps = ps_h.tile([M, HB], F32, tag="h")
                        for k in range(NKCHUNK):
                            nc.tensor.matmul(
                                h_ps[:], wcub_sb[:, k, :], xsq[k][:],
                                start=(k == 0), stop=False,
                            )
                        nc.tensor.matmul(h_ps[:], wx_sb[:], xT_sb[0:D, :], start=False, stop=False)
                        nc.tensor.matmul(h_ps[:], wd32_sb[:], xd32_sb[:], start=False, stop=True)
                        h2_sb = ypool.tile([M, HB], F32, tag=f"h2{h}")
                        nc.vector.tensor_copy(h2_sb[:], h_ps[:])

                        for bc in range(NBH):
                            bs = slice(bc * 128, (bc + 1) * 128)
                            h2t_ps = ps_small.tile([128, M], F32, tag="small")
                            nc.tensor.transpose(h2t_ps[:], h2_sb[:, bs], ident_sb[0:M, 0:M])

                            lin_ps = ps_small.tile([128, out_l], F32, tag="small")
                            nc.tensor.matmul(lin_ps[:], xT_sb[0:65, bs], wlin_sb[:], start=True, stop=True)
                            xmac_ps = ps_small.tile([128, I_PER], F32, tag="small")
                            nc.tensor.matmul(xmac_ps[:], xT_sb[0:D, bs], colsel_sb[:], start=True, stop=True)
                            xmac_sb = ypool.tile([128, I_PER], F32, tag="xmac")
                            nc.scalar.copy(xmac_sb[:], xmac_ps[:])

                            tmp_sb = hpool.tile([128, M + out_l], F32, tag="tmp2")
                            xmac_b = (
                                xmac_sb[:].unsqueeze(2).to_broadcast([128, I_PER, out_l])
                            )
                            nc.vector.tensor_tensor(
                                tmp_sb[:, 0:M].rearrange("p (i o) -> p i o", i=I_PER),
                                h2t_ps[:].rearrange("p (i o) -> p i o", i=I_PER),
                                xmac_b,
                                op=MULT,
                            )
                            nc.scalar.copy(tmp_sb[:, M:], lin_ps[:])
                            nc.vector.tensor_reduce(
                                y_sb[:, bc, :],
                                tmp_sb[:].rearrange("p (i o) -> p o i", i=I_PER + 1),
                                axis=AXIS_X, op=ADD,
                            )

                        nc.sync.dma_start(
                            out_ext.ap()[h * HB:(h + 1) * HB, :]
                            .rearrange("(bc p) o -> p bc o", p=128),
                            y_sb[:],
                        )

                if not last:
                    x_half = next_x

    nc.compile()
    return nc


# ------------------------------------------------------------------- runner --

def kernel(x, W0, b0, W1, b1, W2, b2):
    from concourse.bass_utils import run_bass_kernel_spmd

    if "nc" not in _CACHE:
        _CACHE["nc"] = _build_module()
    nc = _CACHE["nc"]

    x = np.ascontiguousarray(np.asarray(x, np.float32))
    Ws = [np.asarray(W, np.float32) for W in (W0, W1, W2)]
    bs = [np.asarray(b_, np.float32) for b_ in (b0, b1, b2)]

    BT = _u_transform()
    wcubs, wlins = {}, {}
    for li in range(3):
        wcubs[li], wlins[li] = _prep_layer(Ws[li], bs[li], OUTS[li], BT)

    in_maps = []
    for core in range(NCORES):
        I = np.arange(core * I_PER, (core + 1) * I_PER)
        colsel = np.zeros((D, I_PER), np.float16)
        colsel[I, np.arange(I_PER)] = 1.0
        m = {"x": x, "colsel": colsel}
        for li in range(3):
            m[f"wcub{li}"] = wcubs[li][core]
            m[f"wlin{li}"] = wlins[li][core]
        in_maps.append(m)

    res = run_bass_kernel_spmd(nc, in_maps, core_ids=list(range(NCORES)))
    out = np.zeros((B, OUTS[2]), np.float32)
    for core in range(NCORES):
        out += res.results[core]["out"]
    return out
